# revision 11
# baseline (speedup 1.0000x reference)
"""Trainium2 kernel for nn_BernoulliIndependentGenerator.

Full-device pipeline: per-core Bass program computes input projections,
the BiLSTM recurrence (fwd+bwd in one 1024-step hardware loop), and the
gate-score dot products for 2 samples; 8 cores cover B=16 (data
parallel over batch, per the sharding hint). Host does the embedding
gather, input packing, and the final sigmoid + per-row top-k.

Heavy one-time setup (imports, Bass trace, NEFF compile, executable
load, warm-up dispatch) happens at module import; kernel() itself only
packs inputs, runs one dispatch, and post-processes.
"""

import os
import numpy as np

B, S, E, H, V = 16, 1024, 256, 256, 50257
BUDGET = 10
N_CORES = 8
FOUR_H = 4 * H

# ---------------------------------------------------------------------------
# Bass kernel builder (one NeuronCore, SPMD across 8)
# ---------------------------------------------------------------------------
# Core c owns samples a=2c, b=2c+1; sequences u=0..3 = [a-fwd, b-fwd,
# a-bwd, b-bwd]. Gate order permuted to [i, f, o, g]. Weights arrive
# 1/8-partition-sharded and are AllGathered on device. On-chip column
# layouts: psg [128,32] col m*4+u; gates_sb col u*8+m; xpT col
# t*32+u*8+m; c/tmp/tanhc col u*2+k; h_hist col blk*8+u*2+k with fwd
# h[t] at blk t+1 (blk 0 zero-init) and bwd h[t] at blk t+1 (blk S+1
# zero-init; bwd scan step tau processes t = S-1-tau).


def _build_core(S=1024, ag_weights=True):
    import concourse.bass as bass
    import concourse.mybir as mybir
    dt = mybir.dt
    ET = mybir.EngineType
    AF = mybir.ActivationFunctionType
    OP = mybir.AluOpType

    nc = bass.Bass("TRN2")

    KI, KH, M, U = 3, 2, 8, 4
    BS = min(512, S)           # phase-1 token block
    NB = 2 * S // BS           # phase-1 blocks (across both samples)
    CH = min(512, S)           # phase-3 chunk
    NH = S // CH

    WIHC = 2 * KI * M * 128
    WHHC = 2 * KH * M * 128
    CHK = 256                  # tokens per transpose chunk
    NCH = 2 * S // CHK
    emb_in = nc.dram_tensor("emb", [2 * S, 256], dt.float32, kind="ExternalInput")
    vmask_in = nc.dram_tensor("vmask", [1, 2 * S], dt.float32, kind="ExternalInput")
    if ag_weights:
        # 1/8 partition-shard per core; AllGather reconstructs [128, cols]
        wih_in = nc.dram_tensor("wih", [16, WIHC], dt.float32, kind="ExternalInput")
        whh_in = nc.dram_tensor("whh", [16, WHHC], dt.float32, kind="ExternalInput")
        wih_bin = nc.dram_tensor("wih_bin", [16, WIHC], dt.float32)
        wih_bout = nc.dram_tensor("wih_bout", [128, WIHC], dt.float32)
        whh_bin = nc.dram_tensor("whh_bin", [16, WHHC], dt.float32)
        whh_bout = nc.dram_tensor("whh_bout", [128, WHHC], dt.float32)
    else:
        wih_in = nc.dram_tensor("wih", [128, WIHC], dt.float32, kind="ExternalInput")
        whh_in = nc.dram_tensor("whh", [128, WHHC], dt.float32, kind="ExternalInput")
    z_in = nc.dram_tensor("zvec", [128, 132], dt.float32, kind="ExternalInput")
    score_out = nc.dram_tensor("score", [1, 4 * S], dt.float32, kind="ExternalOutput")

    def A(t, off, dims):
        return bass.AP(t, off, [[t.shape[1], 128]] + dims)

    from contextlib import ExitStack
    with ExitStack() as ctx:
        whh_sb = ctx.enter_context(nc.sbuf_tensor("whh_sb", [128, 2 * KH * M * 128], dt.float32))
        z_sb = ctx.enter_context(nc.sbuf_tensor("z_sb", [128, 132], dt.float32))
        xpT = ctx.enter_context(nc.sbuf_tensor("xpT", [128, S * M * U], dt.float32))
        p1ctx = ExitStack()
        emb_sb = p1ctx.enter_context(nc.sbuf_tensor("emb_sb", [128, KI * 2 * S], dt.float32))
        wih_sb = p1ctx.enter_context(nc.sbuf_tensor("wih_sb", [128, 2 * KI * M * 128], dt.float32))
        etok0 = p1ctx.enter_context(nc.sbuf_tensor("etok0", [128, 2 * 256], dt.float32))
        etok1 = p1ctx.enter_context(nc.sbuf_tensor("etok1", [128, 2 * 256], dt.float32))
        ps0 = ctx.enter_context(nc.psum_tensor("ps0", [128, BS], dt.float32))
        ps1 = ctx.enter_context(nc.psum_tensor("ps1", [128, BS], dt.float32))
        ps2 = ctx.enter_context(nc.psum_tensor("ps2", [128, BS], dt.float32))
        ps3 = ctx.enter_context(nc.psum_tensor("ps3", [128, BS], dt.float32))
        psg = ctx.enter_context(nc.psum_tensor("psg", [128, M * U], dt.float32))
        pss0 = ctx.enter_context(nc.psum_tensor("pss0", [1, CH], dt.float32))
        pss1 = ctx.enter_context(nc.psum_tensor("pss1", [1, CH], dt.float32))
        dma_sem = ctx.enter_context(nc.semaphore("dma_sem"))
        m_sem = ctx.enter_context(nc.semaphore("m_sem"))
        pe1_sem = ctx.enter_context(nc.semaphore("pe1_sem"))
        x_sem = ctx.enter_context(nc.semaphore("x_sem"))
        pe_sem = ctx.enter_context(nc.semaphore("pe_sem"))
        g_sem = ctx.enter_context(nc.semaphore("g_sem"))
        act1_sem = ctx.enter_context(nc.semaphore("act1_sem"))
        c_sem = ctx.enter_context(nc.semaphore("c_sem"))
        act2_sem = ctx.enter_context(nc.semaphore("act2_sem"))
        d_sem = ctx.enter_context(nc.semaphore("d_sem"))
        p3_sem = ctx.enter_context(nc.semaphore("p3_sem"))
        tp_sem = ctx.enter_context(nc.semaphore("tp_sem"))
        tc_sem = ctx.enter_context(nc.semaphore("tc_sem"))
        cc_sem = ctx.enter_context(nc.semaphore("cc_sem"))
        s_sem = ctx.enter_context(nc.semaphore("s_sem"))
        o_sem = ctx.enter_context(nc.semaphore("o_sem"))
        ps1_tiles = [ps0, ps1, ps2, ps3]

        # ---------------- input DMAs (SP queue) ----------------
        if ag_weights:
            cc_sem2 = ctx.enter_context(nc.semaphore("cc_sem2"))
            nc.sync.dma_start(wih_bin[:, :], wih_in[:, :]).then_inc(dma_sem, 16)
            nc.sync.dma_start(whh_bin[:, :], whh_in[:, :]).then_inc(dma_sem, 16)
            nc.gpsimd.wait_ge(dma_sem, 48)
            nc.gpsimd.collective_compute(
                "AllGather", mybir.AluOpType.bypass,
                replica_groups=[list(range(8))],
                ins=[wih_bin[:, :]], outs=[wih_bout[:, :]],
            ).then_inc(cc_sem2, 1)
            nc.gpsimd.collective_compute(
                "AllGather", mybir.AluOpType.bypass,
                replica_groups=[list(range(8))],
                ins=[whh_bin[:, :]], outs=[whh_bout[:, :]],
            ).then_inc(cc_sem2, 1)
            nc.sync.wait_ge(cc_sem2, 2)
            nc.sync.dma_start(wih_sb[:, :], wih_bout[:, :]).then_inc(dma_sem, 16)
            nc.sync.dma_start(whh_sb[:, :], whh_bout[:, :]).then_inc(dma_sem, 16)
        else:
            nc.sync.dma_start(wih_sb[:, :], wih_in[:, :]).then_inc(dma_sem, 16)
            nc.sync.dma_start(whh_sb[:, :], whh_in[:, :]).then_inc(dma_sem, 16)
        nc.sync.dma_start(z_sb[:, :], z_in[:, :]).then_inc(dma_sem, 16)
        # vmask lands in partition 0 of the emb k=2 block, after DVE zeroes it
        nc.sync.wait_ge(m_sem, 1)
        nc.sync.dma_start(emb_sb[0:1, 2 * 2 * S:3 * 2 * S], vmask_in[:, :]).then_inc(dma_sem, 16)
        N_DMA_BASE = 6 if ag_weights else 4
        # chunked token-major emb DMAs (double-buffered; device transposes).
        # Both sides iterate (p, j, e): src row = ch*256 + j*128 + p.
        etoks = [etok0, etok1]
        for ch in range(NCH):
            if ch >= 2:
                nc.sync.wait_ge(tc_sem, (ch - 1) * 4)
            dst = bass.AP(etoks[ch % 2], 0, [[512, 128], [256, 2], [1, 256]])
            srcap = bass.AP(emb_in, ch * 256 * 256, [[256, 128], [128 * 256, 2], [1, 256]])
            nc.sync.dma_start(dst, srcap).then_inc(dma_sem, 16)
        N_DMA_IN = N_DMA_BASE + NCH

        # ---------------- DVE setup: memsets ----------------
        nc.vector.memset(emb_sb[:, 2 * 2 * S:3 * 2 * S], 0.0).then_inc(m_sem, 1)

        # ---------------- phase 0.5: transpose emb chunks into emb_sb ----------------
        tp_idx = 0
        for ch in range(NCH):
            nc.tensor.wait_ge(dma_sem, 16 * (N_DMA_BASE + ch + 1))
            for j in range(2):
                for k in range(2):
                    ps = ps1_tiles[tp_idx % 4]
                    if tp_idx >= 4:
                        nc.tensor.wait_ge(tc_sem, tp_idx - 3)
                    nc.tensor.transpose(
                        ps[:, 0:128],
                        etoks[ch % 2][:, j * 256 + k * 128:j * 256 + k * 128 + 128],
                        z_sb[:, 4:132],
                    ).then_inc(tp_sem, 1)
                    nc.vector.wait_ge(tp_sem, tp_idx + 1)
                    nc.vector.tensor_copy(
                        emb_sb[:, k * 2 * S + (ch * 2 + j) * 128:k * 2 * S + (ch * 2 + j) * 128 + 128],
                        ps[:, 0:128],
                    ).then_inc(tc_sem, 1)
                    tp_idx += 1

        # ---------------- phase 1: xp^T = W' @ emb'^T ----------------
        nc.tensor.wait_ge(dma_sem, 16 * N_DMA_IN)
        nc.tensor.wait_ge(m_sem, 1)
        nc.tensor.wait_ge(tc_sem, 4 * NCH)
        g_idx = 0
        for d in range(2):
            for m in range(M):
                for blk in range(NB):
                    ps = ps1_tiles[g_idx % 4]
                    if g_idx >= 4:
                        nc.tensor.wait_ge(x_sem, g_idx - 3)
                    for k in range(KI):
                        mm = nc.tensor.matmul(
                            ps[:, :],
                            wih_sb[:, ((d * KI + k) * M + m) * 128:((d * KI + k) * M + m) * 128 + 128],
                            emb_sb[:, k * 2 * S + blk * BS:k * 2 * S + blk * BS + BS],
                            start=(k == 0),
                            stop=(k == KI - 1),
                        )
                        if k == KI - 1:
                            mm.then_inc(pe1_sem, 1)
                    s = blk // (S // BS)
                    t0 = (blk % (S // BS)) * BS
                    u = 2 * d + s
                    nc.vector.wait_ge(pe1_sem, g_idx + 1)
                    nc.vector.tensor_copy(
                        A(xpT, t0 * 32 + u * 8 + m, [[32, BS]]), ps[:, :]
                    ).then_inc(x_sem, 1)
                    g_idx += 1

        # ---------------- phase 2: recurrence ----------------
        # emb/wih are dead past phase 1; their SBUF is reused for the loop
        # state below. Safe because the DVE memsets sit after the phase-1
        # copies in the DVE stream (past all PE reads via pe1_sem waits),
        # and PE/ACT enter the loop only via d_sem/g_sem which the DVE
        # drives.
        p1ctx.close()
        score_sb = ctx.enter_context(nc.sbuf_tensor("score_sb", [1, 4 * S], dt.float32))
        h_hist = ctx.enter_context(nc.sbuf_tensor("h_hist", [128, (S + 2) * KH * U], dt.float32))
        c_sb = ctx.enter_context(nc.sbuf_tensor("c_sb", [128, KH * U], dt.float32))
        gates_sb = ctx.enter_context(nc.sbuf_tensor("gates_sb", [128, M * U], dt.float32))
        tanhc_sb = ctx.enter_context(nc.sbuf_tensor("tanhc_sb", [128, KH * U], dt.float32))
        tmp_sb = ctx.enter_context(nc.sbuf_tensor("tmp_sb", [128, KH * U], dt.float32))
        tmp2_sb = ctx.enter_context(nc.sbuf_tensor("tmp2_sb", [128, KH * U], dt.float32))
        KU = KH * U
        nc.vector.memset(h_hist[:, 0:KU], 0.0).then_inc(d_sem, 1)
        nc.vector.memset(h_hist[:, (S + 1) * KU:(S + 2) * KU], 0.0).then_inc(d_sem, 1)
        nc.vector.memset(c_sb[:, :], 0.0)
        NG1 = 2 * M * NB
        nc.vector.wait_ge(x_sem, NG1)   # xpT fully materialized
        with nc.Fori(0, S, engines=[ET.PE, ET.DVE, ET.Activation]) as i:
            # PE: psg[m*4+2d : +2] (+)= Whh'(d,k,m) @ h_prev(d,k)
            nc.tensor.wait_ge(d_sem, i * 2 + 2)
            for d in range(2):
                if d == 0:
                    blk_off = i * 8            # fwd reads blk i
                else:
                    blk_off = i * (-8) + (S + 1) * 8   # bwd reads blk S+1-i
                for m in range(M):
                    for k in range(KH):
                        mm = nc.tensor.matmul(
                            psg[:, m * 4 + 2 * d:m * 4 + 2 * d + 2],
                            whh_sb[:, ((d * KH + k) * M + m) * 128:((d * KH + k) * M + m) * 128 + 128],
                            A(h_hist, blk_off + d * 4 + k, [[2, 2]]),
                            start=(k == 0),
                            stop=(k == KH - 1),
                        )
                        if d == 1 and m == M - 1 and k == KH - 1:
                            mm.then_inc(pe_sem, 1)

            # DVE: gates[u,m] = psg[m,u] + xpT[t]
            nc.vector.wait_ge(pe_sem, i + 1)
            nc.vector.tensor_tensor(
                A(gates_sb, 0, [[8, 2], [1, 8]]),
                A(psg, 0, [[1, 2], [4, 8]]),
                A(xpT, i * 32, [[8, 2], [1, 8]]),
                op=OP.add,
            )
            nc.vector.tensor_tensor(
                A(gates_sb, 16, [[8, 2], [1, 8]]),
                A(psg, 2, [[1, 2], [4, 8]]),
                A(xpT, i * (-32) + (S - 1) * 32 + 16, [[8, 2], [1, 8]]),
                op=OP.add,
            ).then_inc(g_sem, 1)

            # ACT: sigmoid(i,f,o) / tanh(g)
            nc.scalar.wait_ge(g_sem, i + 1)
            nc.scalar.activation(
                A(gates_sb, 0, [[8, 4], [1, 6]]), A(gates_sb, 0, [[8, 4], [1, 6]]), AF.Sigmoid
            )
            nc.scalar.activation(
                A(gates_sb, 6, [[8, 4], [1, 2]]), A(gates_sb, 6, [[8, 4], [1, 2]]), AF.Tanh
            ).then_inc(act1_sem, 1)

            # DVE: c = f*c + i*g  (split across buffers; DVE pipe depth 2)
            nc.vector.wait_ge(act1_sem, i + 1)
            nc.vector.tensor_tensor(
                A(tmp2_sb, 0, [[2, 4], [1, 2]]),
                A(c_sb, 0, [[2, 4], [1, 2]]),
                A(gates_sb, 2, [[8, 4], [1, 2]]),
                op=OP.mult,
            ).then_inc(cc_sem, 1)
            nc.vector.tensor_tensor(
                A(tmp_sb, 0, [[2, 4], [1, 2]]),
                A(gates_sb, 0, [[8, 4], [1, 2]]),
                A(gates_sb, 6, [[8, 4], [1, 2]]),
                op=OP.mult,
            ).then_inc(cc_sem, 1)
            nc.vector.wait_ge(cc_sem, i * 2 + 2)
            nc.vector.tensor_tensor(
                c_sb[:, :], tmp2_sb[:, :], tmp_sb[:, :], op=OP.add
            ).then_inc(c_sem, 1)

            # ACT: tanh(c)
            nc.scalar.wait_ge(c_sem, i + 1)
            nc.scalar.activation(tanhc_sb[:, :], c_sb[:, :], AF.Tanh).then_inc(act2_sem, 1)

            # DVE: h = o * tanh(c) -> h_hist
            nc.vector.wait_ge(act2_sem, i + 1)
            nc.vector.tensor_tensor(
                A(h_hist, i * 8 + 8, [[2, 2], [1, 2]]),
                A(gates_sb, 4, [[8, 2], [1, 2]]),
                A(tanhc_sb, 0, [[2, 2], [1, 2]]),
                op=OP.mult,
            ).then_inc(d_sem, 1)
            nc.vector.tensor_tensor(
                A(h_hist, i * (-8) + S * 8 + 4, [[2, 2], [1, 2]]),
                A(gates_sb, 20, [[8, 2], [1, 2]]),
                A(tanhc_sb, 4, [[2, 2], [1, 2]]),
                op=OP.mult,
            ).then_inc(d_sem, 1)

        # ---------------- phase 3: score partials ----------------
        nc.tensor.wait_ge(d_sem, 2 * S + 2)
        pss = [pss0, pss1]
        g3 = 0
        for u in range(U):
            zc = 0 if u < 2 else 2
            for half in range(NH):
                ps = pss[g3 % 2]
                if g3 >= 2:
                    nc.tensor.wait_ge(s_sem, g3 - 1)
                for k in range(KH):
                    mm = nc.tensor.matmul(
                        ps[:, :],
                        z_sb[:, zc + k:zc + k + 1],
                        A(h_hist, (1 + half * CH) * 8 + u * 2 + k, [[8, CH]]),
                        start=(k == 0),
                        stop=(k == KH - 1),
                    )
                    if k == KH - 1:
                        mm.then_inc(p3_sem, 1)
                nc.vector.wait_ge(p3_sem, g3 + 1)
                nc.vector.tensor_copy(
                    score_sb[0:1, u * S + half * CH:u * S + half * CH + CH], ps[:, :]
                ).then_inc(s_sem, 1)
                g3 += 1

        # ---------------- output DMA ----------------
        nc.sync.wait_ge(s_sem, g3)
        nc.sync.dma_start(score_out[:, :], score_sb[:, :]).then_inc(o_sem, 16)
        nc.sync.wait_ge(o_sem, 16)

        # ---------------- teardown ----------------
        gpsimd_type = nc.gpsimd.engine
        for eng_type, eng in nc.engines.items():
            if eng_type == gpsimd_type:
                continue
            dr = mybir.InstDrain(
                name=nc.get_next_instruction_name(), ins=[], outs=[], bass_is_fusable=False,
            )
            dr.engine = eng_type
            eng.add_instruction(dr)
        nc.all_engine_barrier(sem_only=True)

    return nc




# ---------------------------------------------------------------------------
# device setup (import time)
# ---------------------------------------------------------------------------

_DEV = None


def _init_device():
    import jax
    from jax.sharding import Mesh, PartitionSpec
    from jax.experimental.shard_map import shard_map
    import concourse.mybir as mybir
    from concourse.bass2jax import (
        install_neuronx_cc_hook, _bass_exec_p, partition_id_tensor,
    )

    install_neuronx_cc_hook()
    nc = _build_core(S)

    # Strip per-instruction/allocation debug info (absolute file paths +
    # tracebacks) so the serialized BIR — and hence the NEFF cache key — is
    # identical no matter which directory this file runs from.
    for f in nc.m.functions:
        for blk in f.blocks:
            for inst in blk.instructions:
                if getattr(inst, "debug", None) is not None:
                    inst.debug = None
                if getattr(inst, "bass_addl_debug", None) is not None:
                    inst.bass_addl_debug = None
        for alloc in f.allocations:
            for ml in (getattr(alloc, "memorylocations", None) or []):
                ml.ant_debug = None

    pname = nc.partition_id_tensor.name if nc.partition_id_tensor else None
    in_names, out_names, out_avals, zero_outs = [], [], [], []
    for alloc in nc.m.functions[0].allocations:
        if not isinstance(alloc, mybir.MemoryLocationSet):
            continue
        name = alloc.memorylocations[0].name
        if alloc.kind == "ExternalInput":
            if name != pname:
                in_names.append(name)
        elif alloc.kind == "ExternalOutput":
            out_names.append(name)
            shape = tuple(alloc.tensor_shape)
            dtype = mybir.dt.np(alloc.dtype)
            out_avals.append(jax.core.ShapedArray(shape, dtype))
            zero_outs.append((shape, dtype))
    n_params = len(in_names)
    n_outs = len(out_avals)
    in_names_all = list(in_names) + out_names + ([pname] if pname else [])
    donate = tuple(range(n_params, n_params + n_outs))

    def _body(*args):
        operands = list(args)
        if pname is not None:
            operands.append(partition_id_tensor())
        outs = _bass_exec_p.bind(
            *operands, out_avals=tuple(out_avals), in_names=tuple(in_names_all),
            out_names=tuple(out_names), lowering_input_output_aliases=(),
            sim_require_finite=False, sim_require_nnan=False, nc=nc)
        return tuple(outs)

    devices = jax.devices()[:N_CORES]
    mesh = Mesh(np.asarray(devices), ("core",))
    sharded = jax.jit(
        shard_map(_body, mesh=mesh,
                  in_specs=(PartitionSpec("core"),) * (n_params + n_outs),
                  out_specs=(PartitionSpec("core"),) * n_outs, check_rep=False),
        donate_argnums=donate, keep_unused=True)

    shapes = {}
    for alloc in nc.m.functions[0].allocations:
        if not isinstance(alloc, mybir.MemoryLocationSet):
            continue
        name = alloc.memorylocations[0].name
        if name in in_names:
            shapes[name] = tuple(alloc.tensor_shape)

    dev = {
        "sharded": sharded, "in_names": in_names, "zero_outs": zero_outs,
        "in_shapes": shapes,
    }

    # warm-up: compile + load + one dispatch with zeros
    args = [np.zeros((N_CORES * shapes[n][0],) + tuple(shapes[n][1:]), np.float32)
            for n in in_names]
    zo = [np.zeros((N_CORES * sh[0],) + tuple(sh[1:]), dt) for sh, dt in zero_outs]
    res = sharded(*args, *zo)
    np.asarray(res[0])
    return dev


def _run_device(globals_by_name, timeout=60.0):
    """globals_by_name: name -> global [8*rows, cols] array. Returns [8,4,S].
    Runs the dispatch on a worker thread so a wedged device can't hang the
    caller; raises TimeoutError instead."""
    import threading
    dev = _DEV
    args = [globals_by_name[n] for n in dev["in_names"]]
    zo = [np.zeros((N_CORES * sh[0],) + tuple(sh[1:]), dt)
          for sh, dt in dev["zero_outs"]]
    box = {}

    def _work():
        try:
            res = dev["sharded"](*args, *zo)
            box["out"] = np.asarray(res[0])
        except Exception as e:        # noqa: BLE001
            box["err"] = e

    th = threading.Thread(target=_work, daemon=True)
    th.start()
    th.join(timeout)
    if "err" in box:
        raise box["err"]
    if "out" not in box:
        raise TimeoutError("device dispatch timed out")
    return box["out"].reshape(N_CORES, 4, S)


def _try_init(timeout=420.0):
    import threading
    box = {}

    def _work():
        try:
            box["dev"] = _init_device()
        except Exception:             # noqa: BLE001
            box["dev"] = None

    th = threading.Thread(target=_work, daemon=True)
    th.start()
    th.join(timeout)
    return box.get("dev")


if not os.environ.get("KERNEL_NO_DEVICE"):
    _DEV = _try_init()

# ---------------------------------------------------------------------------
# host-side packing
# ---------------------------------------------------------------------------

_PERM = np.concatenate([
    np.arange(0, 256), np.arange(256, 512), np.arange(768, 1024),
    np.arange(512, 768),
])  # torch gate order [i,f,g,o] -> [i,f,o,g]


def _pack_weights(w_ih, b, w_hh):
    Wp = np.zeros((384, 1024), np.float32)
    Wp[:256] = w_ih.T[:, _PERM]
    Wp[256] = b[_PERM]
    Whp = w_hh.T[:, _PERM].astype(np.float32)
    # tiles [128,128], k-major then m: col ((d*K + k)*8 + m)*128 built per dir
    wih = Wp.reshape(3, 128, 8, 128).transpose(1, 0, 2, 3).reshape(128, 3 * 8 * 128)
    whh = Whp.reshape(2, 128, 8, 128).transpose(1, 0, 2, 3).reshape(128, 2 * 8 * 128)
    return np.ascontiguousarray(wih), np.ascontiguousarray(whh)


def _sigmoid(x):
    return 1.0 / (1.0 + np.exp(-x))


def kernel(**inputs):
    x = np.asarray(inputs["x"]).astype(np.int64)
    mask = np.asarray(inputs["mask"]).astype(bool)
    embed_table = np.asarray(inputs["embed_table"], dtype=np.float32)
    w_ih_f = np.asarray(inputs["w_ih_f"], dtype=np.float32)
    w_hh_f = np.asarray(inputs["w_hh_f"], dtype=np.float32)
    b_f = np.asarray(inputs["b_f"], dtype=np.float32)
    w_ih_b = np.asarray(inputs["w_ih_b"], dtype=np.float32)
    w_hh_b = np.asarray(inputs["w_hh_b"], dtype=np.float32)
    b_b = np.asarray(inputs["b_b"], dtype=np.float32)
    z_w = np.asarray(inputs["z_w"], dtype=np.float32)
    z_b = np.float32(np.asarray(inputs["z_b"]))

    lengths = mask.sum(1).astype(np.int64)
    prefix_ok = bool((mask == (np.arange(S)[None, :] < lengths[:, None])).all())

    probs = None
    if _DEV is not None and prefix_ok:
        try:
            probs = _device_probs(x, mask, lengths, embed_table, w_ih_f, w_hh_f,
                                  b_f, w_ih_b, w_hh_b, b_b, z_w, z_b)
        except Exception:
            probs = None
    if probs is None:
        if prefix_ok:
            probs = _host_probs(x, mask, lengths, embed_table, w_ih_f, w_hh_f,
                                b_f, w_ih_b, w_hh_b, b_b, z_w, z_b)
        else:
            probs = _host_probs_masked(x, mask, embed_table, w_ih_f, w_hh_f,
                                       b_f, w_ih_b, w_hh_b, b_b, z_w, z_b)

    probs = np.where(mask, probs, 0.0).astype(np.float32)
    k = np.round(BUDGET / 100.0 * lengths.astype(np.float32)).astype(np.int64)
    ranks = np.argsort(np.argsort(-probs, axis=1, kind="stable"), axis=1, kind="stable")
    z = ((ranks < k[:, None]) & (probs > 0)).astype(np.float32)
    z = np.where(mask, z, 0.0).astype(np.float32)
    return z


def _device_probs(x, mask, lengths, embed_table, w_ih_f, w_hh_f, b_f,
                  w_ih_b, w_hh_b, b_b, z_w, z_b):
    emb = embed_table[x]                        # [B, S, E]
    emb[~mask] = 0.0
    embT = emb.reshape(N_CORES * 2 * S, E)      # token-major; device transposes

    t = np.arange(S)
    vm = (t[None, :] < lengths[:, None]).astype(np.float32)  # [B, S]
    vmask = vm.reshape(N_CORES, 1, 2 * S)

    wih_f, whh_f = _pack_weights(w_ih_f, b_f, w_hh_f)
    wih_b, whh_b = _pack_weights(w_ih_b, b_b, w_hh_b)
    wih = np.concatenate([wih_f, wih_b], axis=1)
    whh = np.concatenate([whh_f, whh_b], axis=1)
    zvec = np.concatenate([
        np.stack([z_w[0:128], z_w[128:256], z_w[256:384], z_w[384:512]], axis=1),
        np.eye(128, dtype=np.float32),
    ], axis=1).astype(np.float32)               # [128, 132], cols 4: identity

    zvec_g = np.empty((N_CORES * 128, 132), np.float32)
    zvec_g.reshape(N_CORES, 128, 132)[:] = zvec

    # wih/whh are 1/8-partition-sharded inputs; the global array is just the
    # packed [128, cols] matrix itself (core c takes rows 16c:16c+16) and the
    # kernel AllGathers on device.
    scores = _run_device({
        "emb": embT, "vmask": vmask.reshape(N_CORES, 2 * S),
        "wih": wih, "whh": whh, "zvec": zvec_g,
    })                                          # [8, 4, S]
    sc = scores.reshape(N_CORES, 2, 2, S)       # [c, dir, s, S]
    score = sc[:, 0] + sc[:, 1]                 # [c, s, S]
    score = score.reshape(B, S) + z_b
    return _sigmoid(score.astype(np.float32))


def _host_probs(x, mask, lengths, embed_table, w_ih_f, w_hh_f, b_f,
                w_ih_b, w_hh_b, b_b, z_w, z_b):
    emb = embed_table[x]
    xp_f = emb @ w_ih_f.T + b_f
    xp_b = emb @ w_ih_b.T + b_b

    h_f = _scan(xp_f, np.ascontiguousarray(w_hh_f.T), reverse=False)

    shift = (S - lengths)
    rows = np.arange(S)[None, :]
    src = rows - shift[:, None]
    src_c = np.clip(src, 0, S - 1)
    gather_idx = src_c[:, :, None]
    xp_b_shifted = np.take_along_axis(
        xp_b, np.broadcast_to(gather_idx, xp_b.shape), axis=1)
    xp_b_shifted = np.where((src >= 0)[:, :, None], xp_b_shifted, 0.0).astype(np.float32)
    h_b_shifted = _scan(xp_b_shifted, np.ascontiguousarray(w_hh_b.T), reverse=True)
    dst = rows + shift[:, None]
    dst_c = np.clip(dst, 0, S - 1)
    h_b = np.take_along_axis(
        h_b_shifted, np.broadcast_to(dst_c[:, :, None], h_b_shifted.shape), axis=1)
    h_b = np.where((dst < S)[:, :, None], h_b, 0.0).astype(np.float32)

    scores = h_f @ z_w[:H] + h_b @ z_w[H:] + z_b
    return _sigmoid(scores.astype(np.float32))


def _scan(xp, w_hh_T, reverse):
    Bn, Sn, _ = xp.shape
    h = np.zeros((Bn, H), np.float32)
    c = np.zeros((Bn, H), np.float32)
    hs = np.empty((Bn, Sn, H), np.float32)
    order = range(Sn - 1, -1, -1) if reverse else range(Sn)
    for t in order:
        gates = xp[:, t, :] + h @ w_hh_T
        i = _sigmoid(gates[:, 0:H])
        f = _sigmoid(gates[:, H:2 * H])
        g = np.tanh(gates[:, 2 * H:3 * H])
        o = _sigmoid(gates[:, 3 * H:4 * H])
        c = f * c + i * g
        h = o * np.tanh(c)
        hs[:, t, :] = h
    return hs


def _host_probs_masked(x, mask, embed_table, w_ih_f, w_hh_f, b_f,
                       w_ih_b, w_hh_b, b_b, z_w, z_b):
    """Exact reference semantics for arbitrary (non-prefix) masks."""
    emb = embed_table[x]
    m = mask.astype(np.float32)[:, :, None]
    out = {}
    for key, (w_ih, bb, w_hh, rev) in {
        "f": (w_ih_f, b_f, w_hh_f, False), "b": (w_ih_b, b_b, w_hh_b, True),
    }.items():
        xp = emb @ w_ih.T + bb
        h = np.zeros((B, H), np.float32)
        c = np.zeros((B, H), np.float32)
        hs = np.empty((B, S, H), np.float32)
        order = range(S - 1, -1, -1) if rev else range(S)
        for t in order:
            gates = xp[:, t, :] + h @ w_hh.T
            i = _sigmoid(gates[:, 0:H]); f = _sigmoid(gates[:, H:2 * H])
            g = np.tanh(gates[:, 2 * H:3 * H]); o = _sigmoid(gates[:, 3 * H:4 * H])
            c_new = f * c + i * g
            h_new = o * np.tanh(c_new)
            mt = m[:, t]
            h = mt * h_new + (1.0 - mt) * h
            c = mt * c_new + (1.0 - mt) * c
            hs[:, t, :] = h * mt
        out[key] = hs
    scores = out["f"] @ z_w[:H] + out["b"] @ z_w[H:] + z_b
    return _sigmoid(scores.astype(np.float32))


# revision 14
# speedup vs baseline: 4.4170x; 4.4170x over previous
"""Trainium2 kernel for nn_BernoulliIndependentGenerator.

Full-device pipeline: per-core Bass program computes input projections,
the BiLSTM recurrence (fwd+bwd in one 1024-step hardware loop), and the
gate-score dot products for 2 samples; 8 cores cover B=16 (data
parallel over batch, per the sharding hint). Host does the embedding
gather, input packing, and the final sigmoid + per-row top-k.

Heavy one-time setup (imports, Bass trace, NEFF compile, executable
load, warm-up dispatch) happens at module import; kernel() itself only
packs inputs, runs one dispatch, and post-processes.
"""

import os
import numpy as np

B, S, E, H, V = 16, 1024, 256, 256, 50257
BUDGET = 10
N_CORES = 8
FOUR_H = 4 * H

# ---------------------------------------------------------------------------
# Bass kernel builder (one NeuronCore, SPMD across 8)
# ---------------------------------------------------------------------------
# Core c owns samples a=2c, b=2c+1; sequences u=0..3 = [a-fwd, b-fwd,
# a-bwd, b-bwd]. Gate order permuted to [i, f, o, g]. Weights arrive
# 1/8-partition-sharded and are AllGathered on device. On-chip column
# layouts: psg [128,32] col m*4+u; gates_sb col u*8+m; xpT col
# t*32+u*8+m; c/tmp/tanhc col u*2+k; h_hist col blk*8+u*2+k with fwd
# h[t] at blk t+1 (blk 0 zero-init) and bwd h[t] at blk t+1 (blk S+1
# zero-init; bwd scan step tau processes t = S-1-tau).


def _build_core(S=1024, ag_weights=True):
    import concourse.bass as bass
    import concourse.mybir as mybir
    dt = mybir.dt
    ET = mybir.EngineType
    AF = mybir.ActivationFunctionType
    OP = mybir.AluOpType

    nc = bass.Bass("TRN2")

    KI, KH, M, U = 3, 2, 8, 4
    BS = min(512, S)           # phase-1 token block
    NB = 2 * S // BS           # phase-1 blocks (across both samples)
    CH = min(512, S)           # phase-3 chunk
    NH = S // CH

    WIHC = 2 * KI * M * 128
    WHHC = 2 * KH * M * 128
    emb_in = nc.dram_tensor("emb", [128, 2 * 2 * S], dt.float32, kind="ExternalInput")
    vmask_in = nc.dram_tensor("vmask", [1, 2 * S], dt.float32, kind="ExternalInput")
    if ag_weights:
        # 1/8 partition-shard per core; AllGather reconstructs [128, cols]
        wih_in = nc.dram_tensor("wih", [16, WIHC], dt.float32, kind="ExternalInput")
        whh_in = nc.dram_tensor("whh", [16, WHHC], dt.float32, kind="ExternalInput")
        wih_bin = nc.dram_tensor("wih_bin", [16, WIHC], dt.float32)
        wih_bout = nc.dram_tensor("wih_bout", [128, WIHC], dt.float32)
        whh_bin = nc.dram_tensor("whh_bin", [16, WHHC], dt.float32)
        whh_bout = nc.dram_tensor("whh_bout", [128, WHHC], dt.float32)
    else:
        wih_in = nc.dram_tensor("wih", [128, WIHC], dt.float32, kind="ExternalInput")
        whh_in = nc.dram_tensor("whh", [128, WHHC], dt.float32, kind="ExternalInput")
    z_in = nc.dram_tensor("zvec", [128, 4], dt.float32, kind="ExternalInput")
    score_out = nc.dram_tensor("score", [1, 4 * S], dt.float32, kind="ExternalOutput")

    def A(t, off, dims):
        return bass.AP(t, off, [[t.shape[1], 128]] + dims)

    from contextlib import ExitStack
    with ExitStack() as ctx:
        whh_sb = ctx.enter_context(nc.sbuf_tensor("whh_sb", [128, 2 * KH * M * 128], dt.float32))
        z_sb = ctx.enter_context(nc.sbuf_tensor("z_sb", [128, 4], dt.float32))
        xpT = ctx.enter_context(nc.sbuf_tensor("xpT", [128, S * M * U], dt.float32))
        p1ctx = ExitStack()
        emb_sb = p1ctx.enter_context(nc.sbuf_tensor("emb_sb", [128, KI * 2 * S], dt.float32))
        wih_sb = p1ctx.enter_context(nc.sbuf_tensor("wih_sb", [128, 2 * KI * M * 128], dt.float32))
        ps0 = ctx.enter_context(nc.psum_tensor("ps0", [128, BS], dt.float32))
        ps1 = ctx.enter_context(nc.psum_tensor("ps1", [128, BS], dt.float32))
        ps2 = ctx.enter_context(nc.psum_tensor("ps2", [128, BS], dt.float32))
        ps3 = ctx.enter_context(nc.psum_tensor("ps3", [128, BS], dt.float32))
        psg = ctx.enter_context(nc.psum_tensor("psg", [128, M * U], dt.float32))
        pss0 = ctx.enter_context(nc.psum_tensor("pss0", [1, CH], dt.float32))
        pss1 = ctx.enter_context(nc.psum_tensor("pss1", [1, CH], dt.float32))
        dma_sem = ctx.enter_context(nc.semaphore("dma_sem"))
        m_sem = ctx.enter_context(nc.semaphore("m_sem"))
        pe1_sem = ctx.enter_context(nc.semaphore("pe1_sem"))
        x_sem = ctx.enter_context(nc.semaphore("x_sem"))
        pe_sem = ctx.enter_context(nc.semaphore("pe_sem"))
        g_sem = ctx.enter_context(nc.semaphore("g_sem"))
        act1_sem = ctx.enter_context(nc.semaphore("act1_sem"))
        c_sem = ctx.enter_context(nc.semaphore("c_sem"))
        act2_sem = ctx.enter_context(nc.semaphore("act2_sem"))
        d_sem = ctx.enter_context(nc.semaphore("d_sem"))
        p3_sem = ctx.enter_context(nc.semaphore("p3_sem"))
        cc_sem = ctx.enter_context(nc.semaphore("cc_sem"))
        s_sem = ctx.enter_context(nc.semaphore("s_sem"))
        o_sem = ctx.enter_context(nc.semaphore("o_sem"))
        ps1_tiles = [ps0, ps1, ps2, ps3]

        # ---------------- input DMAs (SP queue) ----------------
        nc.sync.dma_start(emb_sb[:, 0:2 * 2 * S], emb_in[:, :]).then_inc(dma_sem, 16)
        if ag_weights:
            cc_sem2 = ctx.enter_context(nc.semaphore("cc_sem2"))
            nc.sync.dma_start(wih_bin[:, :], wih_in[:, :]).then_inc(dma_sem, 16)
            nc.sync.dma_start(whh_bin[:, :], whh_in[:, :]).then_inc(dma_sem, 16)
            nc.gpsimd.wait_ge(dma_sem, 48)
            nc.gpsimd.collective_compute(
                "AllGather", mybir.AluOpType.bypass,
                replica_groups=[list(range(8))],
                ins=[wih_bin[:, :]], outs=[wih_bout[:, :]],
            ).then_inc(cc_sem2, 1)
            nc.gpsimd.collective_compute(
                "AllGather", mybir.AluOpType.bypass,
                replica_groups=[list(range(8))],
                ins=[whh_bin[:, :]], outs=[whh_bout[:, :]],
            ).then_inc(cc_sem2, 1)
            nc.sync.wait_ge(cc_sem2, 2)
            nc.sync.dma_start(wih_sb[:, :], wih_bout[:, :]).then_inc(dma_sem, 16)
            nc.sync.dma_start(whh_sb[:, :], whh_bout[:, :]).then_inc(dma_sem, 16)
        else:
            nc.sync.dma_start(wih_sb[:, :], wih_in[:, :]).then_inc(dma_sem, 16)
            nc.sync.dma_start(whh_sb[:, :], whh_in[:, :]).then_inc(dma_sem, 16)
        nc.sync.dma_start(z_sb[:, :], z_in[:, :]).then_inc(dma_sem, 16)
        # vmask lands in partition 0 of the emb k=2 block, after DVE zeroes it
        nc.sync.wait_ge(m_sem, 1)
        nc.sync.dma_start(emb_sb[0:1, 2 * 2 * S:3 * 2 * S], vmask_in[:, :]).then_inc(dma_sem, 16)
        N_DMA_IN = 7 if ag_weights else 5

        # ---------------- DVE setup: memsets ----------------
        nc.vector.memset(emb_sb[:, 2 * 2 * S:3 * 2 * S], 0.0).then_inc(m_sem, 1)

        # ---------------- phase 1: xp^T = W' @ emb'^T ----------------
        nc.tensor.wait_ge(dma_sem, 16 * N_DMA_IN)
        nc.tensor.wait_ge(m_sem, 1)
        g_idx = 0
        for d in range(2):
            for m in range(M):
                for blk in range(NB):
                    ps = ps1_tiles[g_idx % 4]
                    if g_idx >= 4:
                        nc.tensor.wait_ge(x_sem, g_idx - 3)
                    for k in range(KI):
                        mm = nc.tensor.matmul(
                            ps[:, :],
                            wih_sb[:, ((d * KI + k) * M + m) * 128:((d * KI + k) * M + m) * 128 + 128],
                            emb_sb[:, k * 2 * S + blk * BS:k * 2 * S + blk * BS + BS],
                            start=(k == 0),
                            stop=(k == KI - 1),
                        )
                        if k == KI - 1:
                            mm.then_inc(pe1_sem, 1)
                    s = blk // (S // BS)
                    t0 = (blk % (S // BS)) * BS
                    u = 2 * d + s
                    nc.vector.wait_ge(pe1_sem, g_idx + 1)
                    nc.vector.tensor_copy(
                        A(xpT, t0 * 32 + u * 8 + m, [[32, BS]]), ps[:, :]
                    ).then_inc(x_sem, 1)
                    g_idx += 1

        # ---------------- phase 2: recurrence ----------------
        # emb/wih are dead past phase 1; their SBUF is reused for the loop
        # state below. Safe because the DVE memsets sit after the phase-1
        # copies in the DVE stream (past all PE reads via pe1_sem waits),
        # and PE/ACT enter the loop only via d_sem/g_sem which the DVE
        # drives.
        p1ctx.close()
        score_sb = ctx.enter_context(nc.sbuf_tensor("score_sb", [1, 4 * S], dt.float32))
        h_hist = ctx.enter_context(nc.sbuf_tensor("h_hist", [128, (S + 2) * KH * U], dt.float32))
        c_sb = ctx.enter_context(nc.sbuf_tensor("c_sb", [128, KH * U], dt.float32))
        gates_sb = ctx.enter_context(nc.sbuf_tensor("gates_sb", [128, M * U], dt.float32))
        tanhc_sb = ctx.enter_context(nc.sbuf_tensor("tanhc_sb", [128, KH * U], dt.float32))
        tmp_sb = ctx.enter_context(nc.sbuf_tensor("tmp_sb", [128, KH * U], dt.float32))
        tmp2_sb = ctx.enter_context(nc.sbuf_tensor("tmp2_sb", [128, KH * U], dt.float32))
        KU = KH * U
        nc.vector.memset(h_hist[:, 0:KU], 0.0).then_inc(d_sem, 1)
        nc.vector.memset(h_hist[:, (S + 1) * KU:(S + 2) * KU], 0.0).then_inc(d_sem, 1)
        nc.vector.memset(c_sb[:, :], 0.0)
        NG1 = 2 * M * NB
        nc.vector.wait_ge(x_sem, NG1)   # xpT fully materialized
        with nc.Fori(0, S, engines=[ET.PE, ET.DVE, ET.Activation]) as i:
            # PE: psg[m*4+2d : +2] (+)= Whh'(d,k,m) @ h_prev(d,k)
            nc.tensor.wait_ge(d_sem, i * 2 + 2)
            for d in range(2):
                if d == 0:
                    blk_off = i * 8            # fwd reads blk i
                else:
                    blk_off = i * (-8) + (S + 1) * 8   # bwd reads blk S+1-i
                for m in range(M):
                    for k in range(KH):
                        mm = nc.tensor.matmul(
                            psg[:, m * 4 + 2 * d:m * 4 + 2 * d + 2],
                            whh_sb[:, ((d * KH + k) * M + m) * 128:((d * KH + k) * M + m) * 128 + 128],
                            A(h_hist, blk_off + d * 4 + k, [[2, 2]]),
                            start=(k == 0),
                            stop=(k == KH - 1),
                        )
                        if d == 1 and m == M - 1 and k == KH - 1:
                            mm.then_inc(pe_sem, 1)

            # DVE: gates[u,m] = psg[m,u] + xpT[t]
            nc.vector.wait_ge(pe_sem, i + 1)
            nc.vector.tensor_tensor(
                A(gates_sb, 0, [[8, 2], [1, 8]]),
                A(psg, 0, [[1, 2], [4, 8]]),
                A(xpT, i * 32, [[8, 2], [1, 8]]),
                op=OP.add,
            )
            nc.vector.tensor_tensor(
                A(gates_sb, 16, [[8, 2], [1, 8]]),
                A(psg, 2, [[1, 2], [4, 8]]),
                A(xpT, i * (-32) + (S - 1) * 32 + 16, [[8, 2], [1, 8]]),
                op=OP.add,
            ).then_inc(g_sem, 1)

            # ACT: sigmoid(i,f,o) / tanh(g)
            nc.scalar.wait_ge(g_sem, i + 1)
            nc.scalar.activation(
                A(gates_sb, 0, [[8, 4], [1, 6]]), A(gates_sb, 0, [[8, 4], [1, 6]]), AF.Sigmoid
            )
            nc.scalar.activation(
                A(gates_sb, 6, [[8, 4], [1, 2]]), A(gates_sb, 6, [[8, 4], [1, 2]]), AF.Tanh
            ).then_inc(act1_sem, 1)

            # DVE: c = f*c + i*g  (split across buffers; DVE pipe depth 2)
            nc.vector.wait_ge(act1_sem, i + 1)
            nc.vector.tensor_tensor(
                A(tmp2_sb, 0, [[2, 4], [1, 2]]),
                A(c_sb, 0, [[2, 4], [1, 2]]),
                A(gates_sb, 2, [[8, 4], [1, 2]]),
                op=OP.mult,
            ).then_inc(cc_sem, 1)
            nc.vector.tensor_tensor(
                A(tmp_sb, 0, [[2, 4], [1, 2]]),
                A(gates_sb, 0, [[8, 4], [1, 2]]),
                A(gates_sb, 6, [[8, 4], [1, 2]]),
                op=OP.mult,
            ).then_inc(cc_sem, 1)
            nc.vector.wait_ge(cc_sem, i * 2 + 2)
            nc.vector.tensor_tensor(
                c_sb[:, :], tmp2_sb[:, :], tmp_sb[:, :], op=OP.add
            ).then_inc(c_sem, 1)

            # ACT: tanh(c)
            nc.scalar.wait_ge(c_sem, i + 1)
            nc.scalar.activation(tanhc_sb[:, :], c_sb[:, :], AF.Tanh).then_inc(act2_sem, 1)

            # DVE: h = o * tanh(c) -> h_hist
            nc.vector.wait_ge(act2_sem, i + 1)
            nc.vector.tensor_tensor(
                A(h_hist, i * 8 + 8, [[2, 2], [1, 2]]),
                A(gates_sb, 4, [[8, 2], [1, 2]]),
                A(tanhc_sb, 0, [[2, 2], [1, 2]]),
                op=OP.mult,
            ).then_inc(d_sem, 1)
            nc.vector.tensor_tensor(
                A(h_hist, i * (-8) + S * 8 + 4, [[2, 2], [1, 2]]),
                A(gates_sb, 20, [[8, 2], [1, 2]]),
                A(tanhc_sb, 4, [[2, 2], [1, 2]]),
                op=OP.mult,
            ).then_inc(d_sem, 1)

        # ---------------- phase 3: score partials ----------------
        nc.tensor.wait_ge(d_sem, 2 * S + 2)
        pss = [pss0, pss1]
        g3 = 0
        for u in range(U):
            zc = 0 if u < 2 else 2
            for half in range(NH):
                ps = pss[g3 % 2]
                if g3 >= 2:
                    nc.tensor.wait_ge(s_sem, g3 - 1)
                for k in range(KH):
                    mm = nc.tensor.matmul(
                        ps[:, :],
                        z_sb[:, zc + k:zc + k + 1],
                        A(h_hist, (1 + half * CH) * 8 + u * 2 + k, [[8, CH]]),
                        start=(k == 0),
                        stop=(k == KH - 1),
                    )
                    if k == KH - 1:
                        mm.then_inc(p3_sem, 1)
                nc.vector.wait_ge(p3_sem, g3 + 1)
                nc.vector.tensor_copy(
                    score_sb[0:1, u * S + half * CH:u * S + half * CH + CH], ps[:, :]
                ).then_inc(s_sem, 1)
                g3 += 1

        # ---------------- output DMA ----------------
        nc.sync.wait_ge(s_sem, g3)
        nc.sync.dma_start(score_out[:, :], score_sb[:, :]).then_inc(o_sem, 16)
        nc.sync.wait_ge(o_sem, 16)

        # ---------------- teardown ----------------
        gpsimd_type = nc.gpsimd.engine
        for eng_type, eng in nc.engines.items():
            if eng_type == gpsimd_type:
                continue
            dr = mybir.InstDrain(
                name=nc.get_next_instruction_name(), ins=[], outs=[], bass_is_fusable=False,
            )
            dr.engine = eng_type
            eng.add_instruction(dr)
        nc.all_engine_barrier(sem_only=True)

    return nc




# ---------------------------------------------------------------------------
# device setup (import time)
# ---------------------------------------------------------------------------

_DEV = None


def _init_device():
    import jax
    from jax.sharding import Mesh, PartitionSpec
    from jax.experimental.shard_map import shard_map
    import concourse.mybir as mybir
    from concourse.bass2jax import (
        install_neuronx_cc_hook, _bass_exec_p, partition_id_tensor,
    )

    install_neuronx_cc_hook()
    nc = _build_core(S)

    # Strip per-instruction/allocation debug info (absolute file paths +
    # tracebacks) so the serialized BIR — and hence the NEFF cache key — is
    # identical no matter which directory this file runs from.
    for f in nc.m.functions:
        for blk in f.blocks:
            for inst in blk.instructions:
                if getattr(inst, "debug", None) is not None:
                    inst.debug = None
                if getattr(inst, "bass_addl_debug", None) is not None:
                    inst.bass_addl_debug = None
        for alloc in f.allocations:
            for ml in (getattr(alloc, "memorylocations", None) or []):
                ml.ant_debug = None

    pname = nc.partition_id_tensor.name if nc.partition_id_tensor else None
    in_names, out_names, out_avals, zero_outs = [], [], [], []
    for alloc in nc.m.functions[0].allocations:
        if not isinstance(alloc, mybir.MemoryLocationSet):
            continue
        name = alloc.memorylocations[0].name
        if alloc.kind == "ExternalInput":
            if name != pname:
                in_names.append(name)
        elif alloc.kind == "ExternalOutput":
            out_names.append(name)
            shape = tuple(alloc.tensor_shape)
            dtype = mybir.dt.np(alloc.dtype)
            out_avals.append(jax.core.ShapedArray(shape, dtype))
            zero_outs.append((shape, dtype))
    n_params = len(in_names)
    n_outs = len(out_avals)
    in_names_all = list(in_names) + out_names + ([pname] if pname else [])
    donate = tuple(range(n_params, n_params + n_outs))

    def _body(*args):
        operands = list(args)
        if pname is not None:
            operands.append(partition_id_tensor())
        outs = _bass_exec_p.bind(
            *operands, out_avals=tuple(out_avals), in_names=tuple(in_names_all),
            out_names=tuple(out_names), lowering_input_output_aliases=(),
            sim_require_finite=False, sim_require_nnan=False, nc=nc)
        return tuple(outs)

    devices = jax.devices()[:N_CORES]
    mesh = Mesh(np.asarray(devices), ("core",))
    sharded = jax.jit(
        shard_map(_body, mesh=mesh,
                  in_specs=(PartitionSpec("core"),) * (n_params + n_outs),
                  out_specs=(PartitionSpec("core"),) * n_outs, check_rep=False),
        donate_argnums=donate, keep_unused=True)

    shapes = {}
    for alloc in nc.m.functions[0].allocations:
        if not isinstance(alloc, mybir.MemoryLocationSet):
            continue
        name = alloc.memorylocations[0].name
        if name in in_names:
            shapes[name] = tuple(alloc.tensor_shape)

    dev = {
        "sharded": sharded, "in_names": in_names, "zero_outs": zero_outs,
        "in_shapes": shapes,
    }

    # warm-up: compile + load + one dispatch with zeros
    args = [np.zeros((N_CORES * shapes[n][0],) + tuple(shapes[n][1:]), np.float32)
            for n in in_names]
    zo = [np.zeros((N_CORES * sh[0],) + tuple(sh[1:]), dt) for sh, dt in zero_outs]
    res = sharded(*args, *zo)
    np.asarray(res[0])
    return dev


def _run_device(globals_by_name, timeout=60.0):
    """globals_by_name: name -> global [8*rows, cols] array. Returns [8,4,S].
    Runs the dispatch on a worker thread so a wedged device can't hang the
    caller; raises TimeoutError instead."""
    import threading
    dev = _DEV
    args = [globals_by_name[n] for n in dev["in_names"]]
    zo = [np.zeros((N_CORES * sh[0],) + tuple(sh[1:]), dt)
          for sh, dt in dev["zero_outs"]]
    box = {}

    def _work():
        try:
            res = dev["sharded"](*args, *zo)
            box["out"] = np.asarray(res[0])
        except Exception as e:        # noqa: BLE001
            box["err"] = e

    th = threading.Thread(target=_work, daemon=True)
    th.start()
    th.join(timeout)
    if "err" in box:
        raise box["err"]
    if "out" not in box:
        raise TimeoutError("device dispatch timed out")
    return box["out"].reshape(N_CORES, 4, S)


def _try_init(timeout=420.0):
    import threading
    box = {}

    def _work():
        try:
            box["dev"] = _init_device()
        except Exception:             # noqa: BLE001
            box["dev"] = None

    th = threading.Thread(target=_work, daemon=True)
    th.start()
    th.join(timeout)
    return box.get("dev")


if not os.environ.get("KERNEL_NO_DEVICE"):
    _DEV = _try_init()

# ---------------------------------------------------------------------------
# host-side packing
# ---------------------------------------------------------------------------

_PERM = np.concatenate([
    np.arange(0, 256), np.arange(256, 512), np.arange(768, 1024),
    np.arange(512, 768),
])  # torch gate order [i,f,g,o] -> [i,f,o,g]


def _pack_weights(w_ih, b, w_hh):
    Wp = np.zeros((384, 1024), np.float32)
    Wp[:256] = w_ih.T[:, _PERM]
    Wp[256] = b[_PERM]
    Whp = w_hh.T[:, _PERM].astype(np.float32)
    # tiles [128,128], k-major then m: col ((d*K + k)*8 + m)*128 built per dir
    wih = Wp.reshape(3, 128, 8, 128).transpose(1, 0, 2, 3).reshape(128, 3 * 8 * 128)
    whh = Whp.reshape(2, 128, 8, 128).transpose(1, 0, 2, 3).reshape(128, 2 * 8 * 128)
    return np.ascontiguousarray(wih), np.ascontiguousarray(whh)


def _sigmoid(x):
    return 1.0 / (1.0 + np.exp(-x))


def kernel(**inputs):
    x = np.asarray(inputs["x"]).astype(np.int64)
    mask = np.asarray(inputs["mask"]).astype(bool)
    embed_table = np.asarray(inputs["embed_table"], dtype=np.float32)
    w_ih_f = np.asarray(inputs["w_ih_f"], dtype=np.float32)
    w_hh_f = np.asarray(inputs["w_hh_f"], dtype=np.float32)
    b_f = np.asarray(inputs["b_f"], dtype=np.float32)
    w_ih_b = np.asarray(inputs["w_ih_b"], dtype=np.float32)
    w_hh_b = np.asarray(inputs["w_hh_b"], dtype=np.float32)
    b_b = np.asarray(inputs["b_b"], dtype=np.float32)
    z_w = np.asarray(inputs["z_w"], dtype=np.float32)
    z_b = np.float32(np.asarray(inputs["z_b"]))

    lengths = mask.sum(1).astype(np.int64)
    prefix_ok = bool((mask == (np.arange(S)[None, :] < lengths[:, None])).all())

    probs = None
    if _DEV is not None and prefix_ok:
        try:
            probs = _device_probs(x, mask, lengths, embed_table, w_ih_f, w_hh_f,
                                  b_f, w_ih_b, w_hh_b, b_b, z_w, z_b)
        except Exception:
            probs = None
    if probs is None:
        if prefix_ok:
            probs = _host_probs(x, mask, lengths, embed_table, w_ih_f, w_hh_f,
                                b_f, w_ih_b, w_hh_b, b_b, z_w, z_b)
        else:
            probs = _host_probs_masked(x, mask, embed_table, w_ih_f, w_hh_f,
                                       b_f, w_ih_b, w_hh_b, b_b, z_w, z_b)

    probs = np.where(mask, probs, 0.0).astype(np.float32)
    k = np.round(BUDGET / 100.0 * lengths.astype(np.float32)).astype(np.int64)
    ranks = np.argsort(np.argsort(-probs, axis=1, kind="stable"), axis=1, kind="stable")
    z = ((ranks < k[:, None]) & (probs > 0)).astype(np.float32)
    z = np.where(mask, z, 0.0).astype(np.float32)
    return z


def _device_probs(x, mask, lengths, embed_table, w_ih_f, w_hh_f, b_f,
                  w_ih_b, w_hh_b, b_b, z_w, z_b):
    emb = embed_table[x]                        # [B, S, E]
    emb[~mask] = 0.0
    # embT per core: [128, 2*2S]; (c, k, p, s, t)
    embT = np.ascontiguousarray(
        emb.reshape(N_CORES, 2, S, 2, 128).transpose(0, 4, 3, 1, 2)
    ).reshape(N_CORES * 128, 2 * 2 * S)

    t = np.arange(S)
    vm = (t[None, :] < lengths[:, None]).astype(np.float32)  # [B, S]
    vmask = vm.reshape(N_CORES, 1, 2 * S)

    wih_f, whh_f = _pack_weights(w_ih_f, b_f, w_hh_f)
    wih_b, whh_b = _pack_weights(w_ih_b, b_b, w_hh_b)
    wih = np.concatenate([wih_f, wih_b], axis=1)
    whh = np.concatenate([whh_f, whh_b], axis=1)
    zvec = np.ascontiguousarray(
        np.stack([z_w[0:128], z_w[128:256], z_w[256:384], z_w[384:512]], axis=1)
    ).astype(np.float32)

    zvec_g = np.empty((N_CORES * 128, 4), np.float32)
    zvec_g.reshape(N_CORES, 128, 4)[:] = zvec

    # wih/whh are 1/8-partition-sharded inputs; the global array is just the
    # packed [128, cols] matrix itself (core c takes rows 16c:16c+16) and the
    # kernel AllGathers on device.
    scores = _run_device({
        "emb": embT, "vmask": vmask.reshape(N_CORES, 2 * S),
        "wih": wih, "whh": whh, "zvec": zvec_g,
    })                                          # [8, 4, S]
    sc = scores.reshape(N_CORES, 2, 2, S)       # [c, dir, s, S]
    score = sc[:, 0] + sc[:, 1]                 # [c, s, S]
    score = score.reshape(B, S) + z_b
    return _sigmoid(score.astype(np.float32))


def _host_probs(x, mask, lengths, embed_table, w_ih_f, w_hh_f, b_f,
                w_ih_b, w_hh_b, b_b, z_w, z_b):
    emb = embed_table[x]
    xp_f = emb @ w_ih_f.T + b_f
    xp_b = emb @ w_ih_b.T + b_b

    h_f = _scan(xp_f, np.ascontiguousarray(w_hh_f.T), reverse=False)

    shift = (S - lengths)
    rows = np.arange(S)[None, :]
    src = rows - shift[:, None]
    src_c = np.clip(src, 0, S - 1)
    gather_idx = src_c[:, :, None]
    xp_b_shifted = np.take_along_axis(
        xp_b, np.broadcast_to(gather_idx, xp_b.shape), axis=1)
    xp_b_shifted = np.where((src >= 0)[:, :, None], xp_b_shifted, 0.0).astype(np.float32)
    h_b_shifted = _scan(xp_b_shifted, np.ascontiguousarray(w_hh_b.T), reverse=True)
    dst = rows + shift[:, None]
    dst_c = np.clip(dst, 0, S - 1)
    h_b = np.take_along_axis(
        h_b_shifted, np.broadcast_to(dst_c[:, :, None], h_b_shifted.shape), axis=1)
    h_b = np.where((dst < S)[:, :, None], h_b, 0.0).astype(np.float32)

    scores = h_f @ z_w[:H] + h_b @ z_w[H:] + z_b
    return _sigmoid(scores.astype(np.float32))


def _scan(xp, w_hh_T, reverse):
    Bn, Sn, _ = xp.shape
    h = np.zeros((Bn, H), np.float32)
    c = np.zeros((Bn, H), np.float32)
    hs = np.empty((Bn, Sn, H), np.float32)
    order = range(Sn - 1, -1, -1) if reverse else range(Sn)
    for t in order:
        gates = xp[:, t, :] + h @ w_hh_T
        i = _sigmoid(gates[:, 0:H])
        f = _sigmoid(gates[:, H:2 * H])
        g = np.tanh(gates[:, 2 * H:3 * H])
        o = _sigmoid(gates[:, 3 * H:4 * H])
        c = f * c + i * g
        h = o * np.tanh(c)
        hs[:, t, :] = h
    return hs


def _host_probs_masked(x, mask, embed_table, w_ih_f, w_hh_f, b_f,
                       w_ih_b, w_hh_b, b_b, z_w, z_b):
    """Exact reference semantics for arbitrary (non-prefix) masks."""
    emb = embed_table[x]
    m = mask.astype(np.float32)[:, :, None]
    out = {}
    for key, (w_ih, bb, w_hh, rev) in {
        "f": (w_ih_f, b_f, w_hh_f, False), "b": (w_ih_b, b_b, w_hh_b, True),
    }.items():
        xp = emb @ w_ih.T + bb
        h = np.zeros((B, H), np.float32)
        c = np.zeros((B, H), np.float32)
        hs = np.empty((B, S, H), np.float32)
        order = range(S - 1, -1, -1) if rev else range(S)
        for t in order:
            gates = xp[:, t, :] + h @ w_hh.T
            i = _sigmoid(gates[:, 0:H]); f = _sigmoid(gates[:, H:2 * H])
            g = np.tanh(gates[:, 2 * H:3 * H]); o = _sigmoid(gates[:, 3 * H:4 * H])
            c_new = f * c + i * g
            h_new = o * np.tanh(c_new)
            mt = m[:, t]
            h = mt * h_new + (1.0 - mt) * h
            c = mt * c_new + (1.0 - mt) * c
            hs[:, t, :] = h * mt
        out[key] = hs
    scores = out["f"] @ z_w[:H] + out["b"] @ z_w[H:] + z_b
    return _sigmoid(scores.astype(np.float32))
